# revision 37
# baseline (speedup 1.0000x reference)
"""Trainium2 Bass kernel for nn_AdaptiveTransformerModel (gated multi-head
self-attention with per-head scalar normalization), distributed over 8
NeuronCores via head parallelism + AllToAll.

Per-core computation (2 heads, all batches), bf16 matmul path / fp32 stats:
  Q^T/K^T/V^T = (X @ W{q,k,v} + b).T           [128=(2 heads x 64hd), B*T]
  per (batch, q-chunk): S^T = K @ Q^T (row-tiled pair over the 2 heads into
  one 2-bank PSUM tile), E = exp(S^T/8) in one ACT pass, O^T[65] accumulated
  as [V | ones].T @ E (row 64 = softmax denominators), normalize O by the
  denominators (reciprocal_approx_fast + gpsimd broadcast), collect per-token
  sumsq rows to a DRAM bounce. After all batches: batched sqrt -> per-head
  norm means -> s_h = 1/max(mean,1e-5); O *= s_h. AllToAll exchanges
  row-slices so every core gets all 16 heads' O columns for its 1/8 of the
  rows; final matmul P^T = Wo_all^T @ G + bo (gate/H folded into Wo on the
  host). Host gathers P^T slices.
"""
import os
import sys

import numpy as np

for _p in ("/root/.axon_site", "/root/.axon_site/_ro/trn_rl_repo", "/opt/trn_rl_repo"):
    if os.path.isdir(_p) and _p not in sys.path:
        sys.path.append(_p)

import ml_dtypes
import concourse.bass as bass
import concourse.bacc as bacc
import concourse.mybir as mybir
import concourse.tile as tile
from concourse import bass_utils
from concourse.bass import ts
from concourse.masks import make_identity

f32 = mybir.dt.float32
f32r = mybir.dt.float32r
bf16 = mybir.dt.bfloat16
AF = mybir.ActivationFunctionType
ALU = mybir.AluOpType
BF16NP = ml_dtypes.bfloat16

# problem shapes (hardcoded per harness contract)
B, T, D, H = 4, 2048, 1024, 16
HD = 64
NCORES = 8


class Cfg:
    def __init__(self, B=B, T=T, D=D, ncores=NCORES):
        self.B, self.T, self.D, self.ncores = B, T, D, ncores
        self.RT = B * T                  # flattened rows
        self.RSLC = self.RT // ncores    # output row slice per core
        self.DCH = D // 128              # contraction chunks for D
        self.TQ = min(512, self.RSLC, T)  # q-chunk width
        self.NQC = T // self.TQ          # q-chunks per batch
        self.NKT = T // 128              # k-tiles per batch
        self.NRC = self.RT // self.TQ    # TQ-row chunks over all rows
        self.NCH = self.B * self.NQC     # total q-chunks
        assert T % self.TQ == 0 and D % 128 == 0 and self.RT % ncores == 0
        assert self.RSLC % self.TQ == 0


def build_body(ctx, tc, cfg, x, wq, wk, wv, bq, bk, bv, wo, bo, out, dbg=None):
    nc = tc.nc
    DCH, TQ, NKT, NQC, RT = cfg.DCH, cfg.TQ, cfg.NKT, cfg.NQC, cfg.RT

    constp = ctx.enter_context(tc.tile_pool(name="const", bufs=1))
    ident = constp.tile([128, 128], f32)
    make_identity(nc, ident[:])
    ident_bf = constp.tile([128, 128], bf16)
    nc.vector.tensor_copy(ident_bf[:], ident[:])
    ones_f32 = constp.tile([128, 1], f32)
    nc.vector.memset(ones_f32[:], 1.0)
    ones_r = constp.tile([128, 1], f32r)
    nc.vector.tensor_copy(ones_r[:], ones_f32[:])
    ones_bf = constp.tile([128, 1], bf16)
    nc.vector.tensor_copy(ones_bf[:], ones_f32[:])
    bq_sb = constp.tile([128, 1], f32)
    bk_sb = constp.tile([128, 1], f32)
    bv_sb = constp.tile([128, 1], f32)
    nc.sync.dma_start(bq_sb[:], bq[:, None])
    nc.sync.dma_start(bk_sb[:], bk[:, None])
    nc.sync.dma_start(bv_sb[:], bv[:, None])
    bo_sb = constp.tile([128, DCH], f32)
    nc.sync.dma_start(bo_sb[:], bo.rearrange("(c p) -> p c", p=128))

    # persistent buffers (freed LIFO: vaug, kt, qt; o_all lives to the end)
    o_all, free_oa = tc.tile([128, RT], bf16, name="o_all")
    qt_all, free_qt = tc.tile([128, RT], bf16, name="qt_all")
    kt_all, free_kt = tc.tile([128, RT], bf16, name="kt_all")
    vaug, free_va = tc.tile([128, cfg.B, NKT, 2, 65], bf16, name="vaug")
    nc.vector.tensor_copy(
        vaug[:, :, :, :, 64:65],
        ones_f32[:].to_broadcast((128, cfg.B, NKT, 2, 1)))

    dramp = ctx.enter_context(tc.tile_pool(name="dram", bufs=1, space="DRAM"))
    # bounce rows per (head, chunk): softmax sums, sumsq, final scale rows
    nrm_dram = dramp.tile([2, cfg.NCH, TQ], f32)
    sum_dram = dramp.tile([2, cfg.NCH, TQ], f32)
    rs_dram = dramp.tile([2, cfg.NCH, TQ], f32)
    s_dram = dramp.tile([2], f32)

    # ---------------- phase 1: X^T (DMA transpose) + QKV projections --------
    with tc.tile_pool(name="wqkv", bufs=1) as wpool, \
         tc.tile_pool(name="xt", bufs=3) as xtpool, \
         tc.tile_pool(name="vtmp", bufs=2) as vtpool, \
         tc.tile_pool(name="qkvps", bufs=2, space="PSUM") as qkvps, \
         tc.tile_pool(name="vps", bufs=2, space="PSUM") as vps:
        wq_sb = wpool.tile([128, DCH, 128], bf16)
        wk_sb = wpool.tile([128, DCH, 128], bf16)
        wv_sb = wpool.tile([128, DCH, 128], bf16)
        nc.sync.dma_start(wq_sb[:], wq.rearrange("(c p) m -> p c m", p=128))
        nc.sync.dma_start(wk_sb[:], wk.rearrange("(c p) m -> p c m", p=128))
        nc.sync.dma_start(wv_sb[:], wv.rearrange("(c p) m -> p c m", p=128))

        for rc in range(cfg.NRC):           # TQ-row chunks
            r0 = rc * TQ
            xt = xtpool.tile([128, DCH, TQ], bf16)
            for d in range(DCH):
                nc.sync.dma_start_transpose(
                    xt[:, d, :], x[r0:r0 + TQ, ts(d, 128)])
            # Q^T / K^T chunks
            for w_sb, b_sb, dest in ((wq_sb, bq_sb, qt_all), (wk_sb, bk_sb, kt_all)):
                ps = qkvps.tile([128, TQ], f32, tag="qk")
                for d in range(DCH):
                    nc.tensor.matmul(ps[:], w_sb[:, d, :], xt[:, d, :],
                                     start=(d == 0), stop=(d == DCH - 1))
                nc.scalar.activation(dest[:, r0:r0 + TQ], ps[:], AF.Identity,
                                     bias=b_sb[:, 0:1])
            # V chunk -> natural layout tiles with ones column
            ps = qkvps.tile([128, TQ], f32, tag="qk")
            for d in range(DCH):
                nc.tensor.matmul(ps[:], wv_sb[:, d, :], xt[:, d, :],
                                 start=(d == 0), stop=(d == DCH - 1))
            vt = vtpool.tile([128, TQ], bf16)
            nc.scalar.activation(vt[:], ps[:], AF.Identity, bias=bv_sb[:, 0:1])
            b_idx = r0 // cfg.T
            for hl in range(2):
                hs = slice(hl * 64, (hl + 1) * 64)
                for j in range(TQ // 128):
                    vp = vps.tile([128, 64], bf16)
                    nc.tensor.transpose(vp[:], vt[hs, ts(j, 128)],
                                        ident_bf[hs, hs])
                    kt_idx = (r0 % cfg.T) // 128 + j
                    nc.vector.tensor_copy(vaug[:, b_idx, kt_idx, hl, 0:64], vp[:])

    # ---------------- phase 2: attention ----------------
    with tc.tile_pool(name="sps", bufs=2, space="PSUM") as sps, \
         tc.tile_pool(name="ops", bufs=1, space="PSUM") as ops, \
         tc.tile_pool(name="nps", bufs=1, space="PSUM") as nps, \
         tc.tile_pool(name="epool", bufs=3) as epool, \
         tc.tile_pool(name="ph2", bufs=2) as ph2:
        for b in range(cfg.B):
            for qc in range(NQC):
                cc = b * NQC + qc
                c0 = b * cfg.T + qc * TQ
                COLPAIR = True
                o0 = ops.tile([64, TQ], f32, tag="o0")
                nq0 = nps.tile([1, TQ], f32, tag="n0")
                if COLPAIR:
                    o1f = ops.tile([128, TQ], f32, tag="o1", name="o1f")
                    nq1f = nps.tile([33, TQ], f32, tag="n1", name="nq1f")
                    o_out = (o0[:], o1f[64:128, :])
                    nq_out = (nq0[:], nq1f[32:33, :])
                else:
                    o1f = ops.tile([64, TQ], f32, tag="o1", name="o1f")
                    nq1f = nps.tile([1, TQ], f32, tag="n1", name="nq1f")
                    o_out = (o0[:], o1f[:])
                    nq_out = (nq0[:], nq1f[:])
                for t in range(NKT):
                    k0 = b * cfg.T + t * 128
                    s_pair = sps.tile([128, 2 * TQ], f32, tag="s")
                    for hl in range(2):
                        hs = slice(hl * 64, (hl + 1) * 64)
                        nc.tensor.matmul(s_pair[:, ts(hl, TQ)],
                                         kt_all[hs, k0:k0 + 128],
                                         qt_all[hs, c0:c0 + TQ],
                                         start=True, stop=True)
                    e_pair = epool.tile([128, 2 * TQ], bf16, tag="e")
                    nc.scalar.activation(e_pair[:], s_pair[:], AF.Exp, scale=0.125)
                    # col-tiled O pair: head hl output at array cols hl*64..
                    for hl in range(2):
                        nc.tensor.matmul(o_out[hl],
                                         vaug[:, b, t, hl, 0:64],
                                         e_pair[:, ts(hl, TQ)],
                                         start=(t == 0), stop=(t == NKT - 1))
                    # col-paired softmax-sum rows at array cols 0 / 32
                    for hl in range(2):
                        nc.tensor.matmul(nq_out[hl],
                                         ones_bf[:, 0:1],
                                         e_pair[:, ts(hl, TQ)],
                                         start=(t == 0), stop=(t == NKT - 1))
                # drain: store unnormalized O, its squares, and sums rows
                sq = ph2.tile([128, TQ], bf16, tag="sq")
                nc.vector.tensor_copy(o_all[0:64, c0:c0 + TQ], o_out[0])
                nc.vector.tensor_copy(o_all[64:128, c0:c0 + TQ], o_out[1])
                nc.vector.tensor_tensor(out=sq[:, :],
                                        in0=o_all[:, c0:c0 + TQ],
                                        in1=o_all[:, c0:c0 + TQ], op=ALU.mult)
                srow = ph2.tile([128, TQ], f32, tag="srow")
                for hl in range(2):
                    nc.vector.tensor_copy(srow[0:1, :], nq_out[hl])
                    nc.sync.dma_start(sum_dram[hl, cc, :][None, :], srow[0:1, :])
                for hl in range(2):
                    hs = slice(hl * 64, (hl + 1) * 64)
                    nq = nps.tile([1, TQ], f32, tag=f"n{hl}", name="nq")
                    nc.tensor.matmul(nq[:], ones_bf[hs, 0:1],
                                     sq[hs, :], start=True, stop=True)
                    nqs = ph2.tile([128, TQ], f32, tag="nqs")
                    nc.vector.tensor_copy(nqs[0:1, :], nq[:])
                    nc.sync.dma_start(nrm_dram[hl, cc, :][None, :], nqs[0:1, :])

        # batched norm finish: recip of sums, norms, per-head denoms, rs rows
        ncol = cfg.NCH * TQ // 64
        nsb = ph2.tile([128, ncol], f32, tag="nsb")
        ssb = ph2.tile([128, ncol], f32, tag="ssb")
        for hl in range(2):
            nc.sync.dma_start(
                nsb[hl * 64:(hl + 1) * 64, :],
                nrm_dram[hl].rearrange("c q -> (c q)").rearrange(
                    "(p n) -> p n", p=64))
            nc.sync.dma_start(
                ssb[hl * 64:(hl + 1) * 64, :],
                sum_dram[hl].rearrange("c q -> (c q)").rearrange(
                    "(p n) -> p n", p=64))
        recb = ph2.tile([128, ncol], f32, tag="recb")
        nc.vector.reciprocal(recb[:], ssb[:])
        # norm_n[q] = sqrt(sumsq_u[q]) * recip[q]
        nrt = ph2.tile([128, ncol], f32, tag="nrt")
        nc.scalar.activation(nrt[:], nsb[:], AF.Sqrt)
        nc.vector.tensor_tensor(out=nrt[:], in0=nrt[:], in1=recb[:], op=ALU.mult)
        rsum = ph2.tile([128, 1], f32, tag="rsum")
        nc.vector.tensor_reduce(rsum[:], nrt[:], axis=mybir.AxisListType.X,
                                op=ALU.add)
        ntot = nps.tile([1, 2], f32, tag="n0", name="ntot")
        for hl in range(2):
            hs = slice(hl * 64, (hl + 1) * 64)
            nc.tensor.matmul(ntot[0:1, hl:hl + 1], ones_f32[hs, 0:1],
                             rsum[hs, :], start=True, stop=True)
        # per-head scale s_h = 1/max(ntot/RT, 1e-5)
        s_sb = ph2.tile([128, 2], f32, tag="s")
        nc.vector.tensor_scalar(out=s_sb[0:1, :], in0=ntot[0:1, :],
                                scalar1=1.0 / RT, scalar2=1e-5, op0=ALU.mult,
                                op1=ALU.max)
        nc.vector.reciprocal(s_sb[0:1, :], s_sb[0:1, :])
        nc.sync.dma_start(s_dram[:][None, :], s_sb[0:1, :])
        s_vec = ph2.tile([128, 1], f32, tag="sv")
        for hl in range(2):
            nc.sync.dma_start(s_vec[hl * 64:(hl + 1) * 64, :],
                              s_dram[hl:hl + 1][None, :].to_broadcast((64, 1)))
        # rs rows = recip * s_h, bounced to DRAM then broadcast per chunk
        nc.vector.tensor_scalar(out=recb[:], in0=recb[:], scalar1=s_vec[:, 0:1],
                                scalar2=None, op0=ALU.mult)
        for hl in range(2):
            nc.sync.dma_start(
                rs_dram[hl].rearrange("c q -> (c q)").rearrange(
                    "(p n) -> p n", p=64),
                recb[hl * 64:(hl + 1) * 64, :])
        for cc in range(cfg.NCH):
            c0 = cc * TQ
            rb = ph2.tile([128, TQ], f32, tag="rb")
            for hl in range(2):
                nc.sync.dma_start(
                    rb[hl * 64:(hl + 1) * 64, :],
                    rs_dram[hl, cc, :][None, :].to_broadcast((64, TQ)))
            nc.vector.tensor_tensor(out=o_all[:, c0:c0 + TQ],
                                    in0=o_all[:, c0:c0 + TQ], in1=rb[:],
                                    op=ALU.mult)

    free_va()
    free_kt()
    free_qt()

    # ---------------- A2A (split in row-halves) + final projection ----------
    NHALF = cfg.RSLC // 2
    a2a_in = [dramp.tile([cfg.ncores, 128, NHALF], bf16, name=f"a2a_in{h}")
              for h in range(2)]
    a2a_out = [dramp.tile([cfg.ncores, 128, NHALF], bf16, name=f"a2a_out{h}")
               for h in range(2)]
    for h in range(2):
        for s in range(cfg.ncores):
            nc.sync.dma_start(a2a_in[h][s],
                              o_all[:, s * cfg.RSLC + h * NHALF:
                                    s * cfg.RSLC + (h + 1) * NHALF])
        nc.gpsimd.collective_compute(
            "AllToAll", ALU.bypass,
            replica_groups=[list(range(cfg.ncores))],
            ins=[a2a_in[h][:].opt()], outs=[a2a_out[h][:].opt()])
    if dbg is not None:
        nc.sync.dma_start(dbg["dbg_o"], o_all[:])
        for h in range(2):
            nc.sync.dma_start(
                dbg["dbg_a2a"].rearrange("a (h q) -> h a q", h=2)[h],
                a2a_out[h].rearrange("c p q -> (c p) q"))

    HCH = (128 * cfg.ncores) // 128  # hd_all contraction chunks
    with tc.tile_pool(name="wo", bufs=1) as wop, \
         tc.tile_pool(name="g", bufs=2) as gp, \
         tc.tile_pool(name="pps", bufs=2, space="PSUM") as pps, \
         tc.tile_pool(name="pout", bufs=3) as poutp:
        wo_sb = wop.tile([128, HCH, cfg.D], bf16)
        nc.sync.dma_start(wo_sb[:], wo.rearrange("(c p) m -> p c m", p=128))
        FTQ = min(TQ, NHALF)
        for h in range(2):
            g_sb = gp.tile([128, HCH, NHALF], bf16, tag="g")
            nc.sync.dma_start(g_sb[:], a2a_out[h].rearrange("c p q -> p c q"))
            for dsub in range(DCH):
                for rc2 in range(NHALF // FTQ):
                    ps = pps.tile([128, FTQ], f32)
                    for j in range(HCH):
                        nc.tensor.matmul(ps[:], wo_sb[:, j, ts(dsub, 128)],
                                         g_sb[:, j, ts(rc2, FTQ)],
                                         start=(j == 0), stop=(j == HCH - 1))
                    po = poutp.tile([128, FTQ], f32)
                    nc.vector.tensor_scalar(out=po[:], in0=ps[:],
                                            scalar1=bo_sb[:, dsub:dsub + 1],
                                            scalar2=None, op0=ALU.add)
                    nc.sync.dma_start(
                        out[ts(dsub, 128),
                            h * NHALF + rc2 * FTQ:h * NHALF + (rc2 + 1) * FTQ],
                        po[:])


def build_nc(cfg, compile=True, debug_outs=False):
    nc = bacc.Bacc("TRN2", target_bir_lowering=False, debug=False,
                   enable_asserts=False, num_devices=cfg.ncores)
    x = nc.dram_tensor("x", [cfg.RT, cfg.D], bf16, kind="ExternalInput").ap()
    wq = nc.dram_tensor("wq", [cfg.D, 128], bf16, kind="ExternalInput").ap()
    wk = nc.dram_tensor("wk", [cfg.D, 128], bf16, kind="ExternalInput").ap()
    wv = nc.dram_tensor("wv", [cfg.D, 128], bf16, kind="ExternalInput").ap()
    bq = nc.dram_tensor("bq", [128], f32, kind="ExternalInput").ap()
    bk = nc.dram_tensor("bk", [128], f32, kind="ExternalInput").ap()
    bv = nc.dram_tensor("bv", [128], f32, kind="ExternalInput").ap()
    wo = nc.dram_tensor("wo", [128 * cfg.ncores, cfg.D], bf16,
                        kind="ExternalInput").ap()
    bo = nc.dram_tensor("bo", [cfg.D], f32, kind="ExternalInput").ap()
    out = nc.dram_tensor("out", [cfg.D, cfg.RSLC], f32, kind="ExternalOutput").ap()
    dbg = None
    if debug_outs:
        dbg = {
            "dbg_o": nc.dram_tensor("dbg_o", [128, cfg.RT], bf16,
                                    kind="ExternalOutput").ap(),
            "dbg_a2a": nc.dram_tensor("dbg_a2a", [cfg.ncores * 128, cfg.RSLC],
                                      bf16, kind="ExternalOutput").ap(),
        }
    from contextlib import ExitStack
    with tile.TileContext(nc) as tc, ExitStack() as ctx:
        build_body(ctx, tc, cfg, x, wq, wk, wv, bq, bk, bv, wo, bo, out, dbg=dbg)
    if compile:
        nc.compile()
    return nc


def make_in_maps(cfg, inputs, H_total=None):
    """Host-side sharding: per-core input dicts."""
    H_tot = H_total or (2 * cfg.ncores)
    X = np.ascontiguousarray(
        np.asarray(inputs["hidden_states"], np.float32).reshape(cfg.RT, cfg.D)
    ).astype(BF16NP)
    gate_clip = np.clip(np.asarray(inputs["gate"], np.float32), 0.0, 1.0)
    Wo = np.asarray(inputs["Wo"], np.float32)
    bo = np.asarray(inputs["bo"], np.float32)
    wo_all = np.ascontiguousarray(np.concatenate(
        [Wo[h] * (gate_clip[h] / H_tot) for h in range(H_tot)],
        axis=0)).astype(BF16NP)
    bo_sum = (bo * (gate_clip[:, None] / H_tot)).sum(axis=0).astype(np.float32)
    in_maps = []
    for c in range(cfg.ncores):
        h0, h1 = 2 * c, 2 * c + 1
        m = {
            "x": X,
            "wq": np.concatenate([inputs["Wq"][h0], inputs["Wq"][h1]], axis=1,
                                 dtype=np.float32).astype(BF16NP),
            "wk": np.concatenate([inputs["Wk"][h0], inputs["Wk"][h1]], axis=1,
                                 dtype=np.float32).astype(BF16NP),
            "wv": np.concatenate([inputs["Wv"][h0], inputs["Wv"][h1]], axis=1,
                                 dtype=np.float32).astype(BF16NP),
            "bq": np.concatenate([inputs["bq"][h0], inputs["bq"][h1]],
                                 dtype=np.float32),
            "bk": np.concatenate([inputs["bk"][h0], inputs["bk"][h1]],
                                 dtype=np.float32),
            "bv": np.concatenate([inputs["bv"][h0], inputs["bv"][h1]],
                                 dtype=np.float32),
            "wo": wo_all,
            "bo": bo_sum,
        }
        in_maps.append(m)
    return in_maps


def gather_out(cfg, results):
    """results: list of per-core out_maps -> full [B, T, D]."""
    parts = [np.asarray(r["out"]).T for r in results]  # each [RSLC, D]
    return np.concatenate(parts, axis=0).reshape(cfg.B, cfg.T, cfg.D)


_COMPILED = {}


def kernel(**inputs) -> np.ndarray:
    cfg = Cfg()
    key = "full"
    if key not in _COMPILED:
        _COMPILED[key] = build_nc(cfg)
    nc = _COMPILED[key]
    in_maps = make_in_maps(cfg, inputs)
    res = bass_utils.run_bass_kernel_spmd(nc, in_maps,
                                          core_ids=list(range(cfg.ncores)))
    return gather_out(cfg, res.results)


if __name__ == "__main__":
    import reference
    inputs = {k: np.asarray(v) for k, v in reference.setup_inputs().items()}
    out = kernel(**inputs)
    exp = np.asarray(reference.reference(**inputs))
    rel = np.linalg.norm(out - exp) / np.linalg.norm(exp)
    print("Relative error:", rel)


# revision 38
# speedup vs baseline: 1.3751x; 1.3751x over previous
"""Trainium2 Bass kernel for nn_AdaptiveTransformerModel (gated multi-head
self-attention with per-head scalar normalization), distributed over 8
NeuronCores via head parallelism + AllToAll.

Per-core computation (2 heads, all batches), bf16 matmul path / fp32 stats:
  Q^T/K^T/V^T = (X @ W{q,k,v} + b).T           [128=(2 heads x 64hd), B*T]
  per (batch, q-chunk): S^T = K @ Q^T (row-tiled pair over the 2 heads into
  one 2-bank PSUM tile), E = exp(S^T/8) in one ACT pass, O^T[65] accumulated
  as [V | ones].T @ E (row 64 = softmax denominators), normalize O by the
  denominators (reciprocal_approx_fast + gpsimd broadcast), collect per-token
  sumsq rows to a DRAM bounce. After all batches: batched sqrt -> per-head
  norm means -> s_h = 1/max(mean,1e-5); O *= s_h. AllToAll exchanges
  row-slices so every core gets all 16 heads' O columns for its 1/8 of the
  rows; final matmul P^T = Wo_all^T @ G + bo (gate/H folded into Wo on the
  host). Host gathers P^T slices.
"""
import os
import sys

import numpy as np

for _p in ("/root/.axon_site", "/root/.axon_site/_ro/trn_rl_repo", "/opt/trn_rl_repo"):
    if os.path.isdir(_p) and _p not in sys.path:
        sys.path.append(_p)

import ml_dtypes
import concourse.bass as bass
import concourse.bacc as bacc
import concourse.mybir as mybir
import concourse.tile as tile
from concourse import bass_utils
from concourse.bass import ts
from concourse.masks import make_identity

f32 = mybir.dt.float32
f32r = mybir.dt.float32r
bf16 = mybir.dt.bfloat16
AF = mybir.ActivationFunctionType
ALU = mybir.AluOpType
BF16NP = ml_dtypes.bfloat16

# problem shapes (hardcoded per harness contract)
B, T, D, H = 4, 2048, 1024, 16
HD = 64
NCORES = 8


class Cfg:
    def __init__(self, B=B, T=T, D=D, ncores=NCORES):
        self.B, self.T, self.D, self.ncores = B, T, D, ncores
        self.RT = B * T                  # flattened rows
        self.RSLC = self.RT // ncores    # output row slice per core
        self.DCH = D // 128              # contraction chunks for D
        self.TQ = min(512, self.RSLC, T)  # q-chunk width
        self.NQC = T // self.TQ          # q-chunks per batch
        self.NKT = T // 128              # k-tiles per batch
        self.NRC = self.RT // self.TQ    # TQ-row chunks over all rows
        self.NCH = self.B * self.NQC     # total q-chunks
        assert T % self.TQ == 0 and D % 128 == 0 and self.RT % ncores == 0
        assert self.RSLC % self.TQ == 0


def build_body(ctx, tc, cfg, x, wq, wk, wv, bq, bk, bv, wo, bo, out, dbg=None):
    nc = tc.nc
    DCH, TQ, NKT, NQC, RT = cfg.DCH, cfg.TQ, cfg.NKT, cfg.NQC, cfg.RT

    constp = ctx.enter_context(tc.tile_pool(name="const", bufs=1))
    ident = constp.tile([128, 128], f32)
    make_identity(nc, ident[:])
    ident_bf = constp.tile([128, 128], bf16)
    nc.vector.tensor_copy(ident_bf[:], ident[:])
    ones_f32 = constp.tile([128, 1], f32)
    nc.vector.memset(ones_f32[:], 1.0)
    ones_r = constp.tile([128, 1], f32r)
    nc.vector.tensor_copy(ones_r[:], ones_f32[:])
    ones_bf = constp.tile([128, 1], bf16)
    nc.vector.tensor_copy(ones_bf[:], ones_f32[:])
    bq_sb = constp.tile([128, 1], f32)
    bk_sb = constp.tile([128, 1], f32)
    bv_sb = constp.tile([128, 1], f32)
    nc.sync.dma_start(bq_sb[:], bq[:, None])
    nc.sync.dma_start(bk_sb[:], bk[:, None])
    nc.sync.dma_start(bv_sb[:], bv[:, None])
    bo_sb = constp.tile([128, DCH], f32)
    nc.sync.dma_start(bo_sb[:], bo.rearrange("(c p) -> p c", p=128))

    # persistent buffers (freed LIFO: vaug, kt, qt; o_all lives to the end)
    o_all, free_oa = tc.tile([128, RT], bf16, name="o_all")
    qt_all, free_qt = tc.tile([128, RT], bf16, name="qt_all")
    kt_all, free_kt = tc.tile([128, RT], bf16, name="kt_all")
    vaug, free_va = tc.tile([128, cfg.B, NKT, 2, 65], bf16, name="vaug")
    nc.vector.tensor_copy(
        vaug[:, :, :, :, 64:65],
        ones_f32[:].to_broadcast((128, cfg.B, NKT, 2, 1)))

    dramp = ctx.enter_context(tc.tile_pool(name="dram", bufs=1, space="DRAM"))
    # bounce rows per (head, chunk): softmax sums, sumsq, final scale rows
    nrm_dram = dramp.tile([2, cfg.NCH, TQ], f32)
    sum_dram = dramp.tile([2, cfg.NCH, TQ], f32)
    rs_dram = dramp.tile([2, cfg.NCH, TQ], f32)
    s_dram = dramp.tile([2], f32)

    # ---------------- phase 1: X^T (DMA transpose) + QKV projections --------
    with tc.tile_pool(name="wqkv", bufs=1) as wpool, \
         tc.tile_pool(name="xt", bufs=3) as xtpool, \
         tc.tile_pool(name="vtmp", bufs=2) as vtpool, \
         tc.tile_pool(name="qkvps", bufs=2, space="PSUM") as qkvps, \
         tc.tile_pool(name="vps", bufs=2, space="PSUM") as vps:
        wq_sb = wpool.tile([128, DCH, 128], bf16)
        wk_sb = wpool.tile([128, DCH, 128], bf16)
        wv_sb = wpool.tile([128, DCH, 128], bf16)
        nc.sync.dma_start(wq_sb[:], wq.rearrange("(c p) m -> p c m", p=128))
        nc.sync.dma_start(wk_sb[:], wk.rearrange("(c p) m -> p c m", p=128))
        nc.sync.dma_start(wv_sb[:], wv.rearrange("(c p) m -> p c m", p=128))

        for rc in range(cfg.NRC):           # TQ-row chunks
            r0 = rc * TQ
            xt = xtpool.tile([128, DCH, TQ], bf16)
            for d in range(DCH):
                nc.sync.dma_start_transpose(
                    xt[:, d, :], x[r0:r0 + TQ, ts(d, 128)])
            # Q^T / K^T chunks
            for w_sb, b_sb, dest in ((wq_sb, bq_sb, qt_all), (wk_sb, bk_sb, kt_all)):
                ps = qkvps.tile([128, TQ], f32, tag="qk")
                for d in range(DCH):
                    nc.tensor.matmul(ps[:], w_sb[:, d, :], xt[:, d, :],
                                     start=(d == 0), stop=(d == DCH - 1))
                nc.scalar.activation(dest[:, r0:r0 + TQ], ps[:], AF.Identity,
                                     bias=b_sb[:, 0:1])
            # V chunk -> natural layout tiles with ones column
            ps = qkvps.tile([128, TQ], f32, tag="qk")
            for d in range(DCH):
                nc.tensor.matmul(ps[:], wv_sb[:, d, :], xt[:, d, :],
                                 start=(d == 0), stop=(d == DCH - 1))
            vt = vtpool.tile([128, TQ], bf16)
            nc.scalar.activation(vt[:], ps[:], AF.Identity, bias=bv_sb[:, 0:1])
            b_idx = r0 // cfg.T
            for hl in range(2):
                hs = slice(hl * 64, (hl + 1) * 64)
                for j in range(TQ // 128):
                    vp = vps.tile([128, 64], bf16)
                    nc.tensor.transpose(vp[:], vt[hs, ts(j, 128)],
                                        ident_bf[hs, hs])
                    kt_idx = (r0 % cfg.T) // 128 + j
                    nc.vector.tensor_copy(vaug[:, b_idx, kt_idx, hl, 0:64], vp[:])

    # ---------------- phase 2: attention ----------------
    with tc.tile_pool(name="sps", bufs=2, space="PSUM") as sps, \
         tc.tile_pool(name="ops", bufs=1, space="PSUM") as ops, \
         tc.tile_pool(name="nps", bufs=1, space="PSUM") as nps, \
         tc.tile_pool(name="epool", bufs=3) as epool, \
         tc.tile_pool(name="ph2", bufs=2) as ph2:
        for b in range(cfg.B):
            for qc in range(NQC):
                cc = b * NQC + qc
                c0 = b * cfg.T + qc * TQ
                o_ps = [ops.tile([65, TQ], f32, tag=f"o{hl}", name=f"o_ps{hl}")
                        for hl in range(2)]
                for t in range(NKT):
                    k0 = b * cfg.T + t * 128
                    s_pair = sps.tile([128, 2 * TQ], f32, tag="s")
                    for hl in range(2):
                        hs = slice(hl * 64, (hl + 1) * 64)
                        nc.tensor.matmul(s_pair[:, ts(hl, TQ)],
                                         kt_all[hs, k0:k0 + 128],
                                         qt_all[hs, c0:c0 + TQ],
                                         start=True, stop=True)
                    e_pair = epool.tile([128, 2 * TQ], bf16, tag="e")
                    nc.scalar.activation(e_pair[:], s_pair[:], AF.Exp, scale=0.125)
                    for hl in range(2):
                        nc.tensor.matmul(o_ps[hl][:], vaug[:, b, t, hl, :],
                                         e_pair[:, ts(hl, TQ)],
                                         start=(t == 0), stop=(t == NKT - 1))
                # drain: store unnormalized O, its squares, and sums rows
                sq = ph2.tile([128, TQ], bf16, tag="sq")
                nc.vector.tensor_copy(o_all[0:64, c0:c0 + TQ], o_ps[0][0:64, :])
                nc.vector.tensor_copy(o_all[64:128, c0:c0 + TQ], o_ps[1][0:64, :])
                nc.vector.tensor_tensor(out=sq[:, :],
                                        in0=o_all[:, c0:c0 + TQ],
                                        in1=o_all[:, c0:c0 + TQ], op=ALU.mult)
                srow = ph2.tile([128, TQ], f32, tag="srow")
                for hl in range(2):
                    nc.vector.tensor_copy(srow[0:1, :], o_ps[hl][64:65, :])
                    nc.sync.dma_start(sum_dram[hl, cc, :][None, :], srow[0:1, :])
                for hl in range(2):
                    hs = slice(hl * 64, (hl + 1) * 64)
                    nq = nps.tile([1, TQ], f32, tag=f"n{hl}", name="nq")
                    nc.tensor.matmul(nq[:], ones_bf[hs, 0:1],
                                     sq[hs, :], start=True, stop=True)
                    nqs = ph2.tile([128, TQ], f32, tag="nqs")
                    nc.vector.tensor_copy(nqs[0:1, :], nq[:])
                    nc.sync.dma_start(nrm_dram[hl, cc, :][None, :], nqs[0:1, :])

        # batched norm finish: recip of sums, norms, per-head denoms, rs rows
        ncol = cfg.NCH * TQ // 64
        nsb = ph2.tile([128, ncol], f32, tag="nsb")
        ssb = ph2.tile([128, ncol], f32, tag="ssb")
        for hl in range(2):
            nc.sync.dma_start(
                nsb[hl * 64:(hl + 1) * 64, :],
                nrm_dram[hl].rearrange("c q -> (c q)").rearrange(
                    "(p n) -> p n", p=64))
            nc.sync.dma_start(
                ssb[hl * 64:(hl + 1) * 64, :],
                sum_dram[hl].rearrange("c q -> (c q)").rearrange(
                    "(p n) -> p n", p=64))
        recb = ph2.tile([128, ncol], f32, tag="recb")
        nc.vector.reciprocal(recb[:], ssb[:])
        # norm_n[q] = sqrt(sumsq_u[q]) * recip[q]
        nrt = ph2.tile([128, ncol], f32, tag="nrt")
        nc.scalar.activation(nrt[:], nsb[:], AF.Sqrt)
        nc.vector.tensor_tensor(out=nrt[:], in0=nrt[:], in1=recb[:], op=ALU.mult)
        rsum = ph2.tile([128, 1], f32, tag="rsum")
        nc.vector.tensor_reduce(rsum[:], nrt[:], axis=mybir.AxisListType.X,
                                op=ALU.add)
        ntot = nps.tile([1, 2], f32, tag="n0", name="ntot")
        for hl in range(2):
            hs = slice(hl * 64, (hl + 1) * 64)
            nc.tensor.matmul(ntot[0:1, hl:hl + 1], ones_f32[hs, 0:1],
                             rsum[hs, :], start=True, stop=True)
        # per-head scale s_h = 1/max(ntot/RT, 1e-5)
        s_sb = ph2.tile([128, 2], f32, tag="s")
        nc.vector.tensor_scalar(out=s_sb[0:1, :], in0=ntot[0:1, :],
                                scalar1=1.0 / RT, scalar2=1e-5, op0=ALU.mult,
                                op1=ALU.max)
        nc.vector.reciprocal(s_sb[0:1, :], s_sb[0:1, :])
        nc.sync.dma_start(s_dram[:][None, :], s_sb[0:1, :])
        s_vec = ph2.tile([128, 1], f32, tag="sv")
        for hl in range(2):
            nc.sync.dma_start(s_vec[hl * 64:(hl + 1) * 64, :],
                              s_dram[hl:hl + 1][None, :].to_broadcast((64, 1)))
        # rs rows = recip * s_h, bounced to DRAM then broadcast per chunk
        nc.vector.tensor_scalar(out=recb[:], in0=recb[:], scalar1=s_vec[:, 0:1],
                                scalar2=None, op0=ALU.mult)
        for hl in range(2):
            nc.sync.dma_start(
                rs_dram[hl].rearrange("c q -> (c q)").rearrange(
                    "(p n) -> p n", p=64),
                recb[hl * 64:(hl + 1) * 64, :])
        for cc in range(cfg.NCH):
            c0 = cc * TQ
            rb = ph2.tile([128, TQ], f32, tag="rb")
            for hl in range(2):
                nc.sync.dma_start(
                    rb[hl * 64:(hl + 1) * 64, :],
                    rs_dram[hl, cc, :][None, :].to_broadcast((64, TQ)))
            nc.vector.tensor_tensor(out=o_all[:, c0:c0 + TQ],
                                    in0=o_all[:, c0:c0 + TQ], in1=rb[:],
                                    op=ALU.mult)

    free_va()
    free_kt()
    free_qt()

    # ---------------- A2A (split in row-halves) + final projection ----------
    NHALF = cfg.RSLC // 2
    a2a_in = [dramp.tile([cfg.ncores, 128, NHALF], bf16, name=f"a2a_in{h}")
              for h in range(2)]
    a2a_out = [dramp.tile([cfg.ncores, 128, NHALF], bf16, name=f"a2a_out{h}")
               for h in range(2)]
    for h in range(2):
        for s in range(cfg.ncores):
            nc.sync.dma_start(a2a_in[h][s],
                              o_all[:, s * cfg.RSLC + h * NHALF:
                                    s * cfg.RSLC + (h + 1) * NHALF])
        nc.gpsimd.collective_compute(
            "AllToAll", ALU.bypass,
            replica_groups=[list(range(cfg.ncores))],
            ins=[a2a_in[h][:].opt()], outs=[a2a_out[h][:].opt()])
    if dbg is not None:
        nc.sync.dma_start(dbg["dbg_o"], o_all[:])
        for h in range(2):
            nc.sync.dma_start(
                dbg["dbg_a2a"].rearrange("a (h q) -> h a q", h=2)[h],
                a2a_out[h].rearrange("c p q -> (c p) q"))

    HCH = (128 * cfg.ncores) // 128  # hd_all contraction chunks
    with tc.tile_pool(name="wo", bufs=1) as wop, \
         tc.tile_pool(name="g", bufs=2) as gp, \
         tc.tile_pool(name="pps", bufs=2, space="PSUM") as pps, \
         tc.tile_pool(name="pout", bufs=3) as poutp:
        wo_sb = wop.tile([128, HCH, cfg.D], bf16)
        nc.sync.dma_start(wo_sb[:], wo.rearrange("(c p) m -> p c m", p=128))
        FTQ = min(TQ, NHALF)
        for h in range(2):
            g_sb = gp.tile([128, HCH, NHALF], bf16, tag="g")
            nc.sync.dma_start(g_sb[:], a2a_out[h].rearrange("c p q -> p c q"))
            for dsub in range(DCH):
                for rc2 in range(NHALF // FTQ):
                    ps = pps.tile([128, FTQ], f32)
                    for j in range(HCH):
                        nc.tensor.matmul(ps[:], wo_sb[:, j, ts(dsub, 128)],
                                         g_sb[:, j, ts(rc2, FTQ)],
                                         start=(j == 0), stop=(j == HCH - 1))
                    po = poutp.tile([128, FTQ], f32)
                    nc.vector.tensor_scalar(out=po[:], in0=ps[:],
                                            scalar1=bo_sb[:, dsub:dsub + 1],
                                            scalar2=None, op0=ALU.add)
                    nc.sync.dma_start(
                        out[ts(dsub, 128),
                            h * NHALF + rc2 * FTQ:h * NHALF + (rc2 + 1) * FTQ],
                        po[:])


def build_nc(cfg, compile=True, debug_outs=False):
    nc = bacc.Bacc("TRN2", target_bir_lowering=False, debug=False,
                   enable_asserts=False, num_devices=cfg.ncores)
    x = nc.dram_tensor("x", [cfg.RT, cfg.D], bf16, kind="ExternalInput").ap()
    wq = nc.dram_tensor("wq", [cfg.D, 128], bf16, kind="ExternalInput").ap()
    wk = nc.dram_tensor("wk", [cfg.D, 128], bf16, kind="ExternalInput").ap()
    wv = nc.dram_tensor("wv", [cfg.D, 128], bf16, kind="ExternalInput").ap()
    bq = nc.dram_tensor("bq", [128], f32, kind="ExternalInput").ap()
    bk = nc.dram_tensor("bk", [128], f32, kind="ExternalInput").ap()
    bv = nc.dram_tensor("bv", [128], f32, kind="ExternalInput").ap()
    wo = nc.dram_tensor("wo", [128 * cfg.ncores, cfg.D], bf16,
                        kind="ExternalInput").ap()
    bo = nc.dram_tensor("bo", [cfg.D], f32, kind="ExternalInput").ap()
    out = nc.dram_tensor("out", [cfg.D, cfg.RSLC], f32, kind="ExternalOutput").ap()
    dbg = None
    if debug_outs:
        dbg = {
            "dbg_o": nc.dram_tensor("dbg_o", [128, cfg.RT], bf16,
                                    kind="ExternalOutput").ap(),
            "dbg_a2a": nc.dram_tensor("dbg_a2a", [cfg.ncores * 128, cfg.RSLC],
                                      bf16, kind="ExternalOutput").ap(),
        }
    from contextlib import ExitStack
    with tile.TileContext(nc) as tc, ExitStack() as ctx:
        build_body(ctx, tc, cfg, x, wq, wk, wv, bq, bk, bv, wo, bo, out, dbg=dbg)
    if compile:
        nc.compile()
    return nc


def make_in_maps(cfg, inputs, H_total=None):
    """Host-side sharding: per-core input dicts."""
    H_tot = H_total or (2 * cfg.ncores)
    X = np.ascontiguousarray(
        np.asarray(inputs["hidden_states"], np.float32).reshape(cfg.RT, cfg.D)
    ).astype(BF16NP)
    gate_clip = np.clip(np.asarray(inputs["gate"], np.float32), 0.0, 1.0)
    Wo = np.asarray(inputs["Wo"], np.float32)
    bo = np.asarray(inputs["bo"], np.float32)
    wo_all = np.ascontiguousarray(np.concatenate(
        [Wo[h] * (gate_clip[h] / H_tot) for h in range(H_tot)],
        axis=0)).astype(BF16NP)
    bo_sum = (bo * (gate_clip[:, None] / H_tot)).sum(axis=0).astype(np.float32)
    in_maps = []
    for c in range(cfg.ncores):
        h0, h1 = 2 * c, 2 * c + 1
        m = {
            "x": X,
            "wq": np.concatenate([inputs["Wq"][h0], inputs["Wq"][h1]], axis=1,
                                 dtype=np.float32).astype(BF16NP),
            "wk": np.concatenate([inputs["Wk"][h0], inputs["Wk"][h1]], axis=1,
                                 dtype=np.float32).astype(BF16NP),
            "wv": np.concatenate([inputs["Wv"][h0], inputs["Wv"][h1]], axis=1,
                                 dtype=np.float32).astype(BF16NP),
            "bq": np.concatenate([inputs["bq"][h0], inputs["bq"][h1]],
                                 dtype=np.float32),
            "bk": np.concatenate([inputs["bk"][h0], inputs["bk"][h1]],
                                 dtype=np.float32),
            "bv": np.concatenate([inputs["bv"][h0], inputs["bv"][h1]],
                                 dtype=np.float32),
            "wo": wo_all,
            "bo": bo_sum,
        }
        in_maps.append(m)
    return in_maps


def gather_out(cfg, results):
    """results: list of per-core out_maps -> full [B, T, D]."""
    parts = [np.asarray(r["out"]).T for r in results]  # each [RSLC, D]
    return np.concatenate(parts, axis=0).reshape(cfg.B, cfg.T, cfg.D)


_COMPILED = {}


def kernel(**inputs) -> np.ndarray:
    cfg = Cfg()
    key = "full"
    if key not in _COMPILED:
        _COMPILED[key] = build_nc(cfg)
    nc = _COMPILED[key]
    in_maps = make_in_maps(cfg, inputs)
    res = bass_utils.run_bass_kernel_spmd(nc, in_maps,
                                          core_ids=list(range(cfg.ncores)))
    return gather_out(cfg, res.results)


if __name__ == "__main__":
    import reference
    inputs = {k: np.asarray(v) for k, v in reference.setup_inputs().items()}
    out = kernel(**inputs)
    exp = np.asarray(reference.reference(**inputs))
    rel = np.linalg.norm(out - exp) / np.linalg.norm(exp)
    print("Relative error:", rel)
